# revision 47
# baseline (speedup 1.0000x reference)
"""CausalSelfAttention Trainium2 kernel (B=2, T=2048, C=1024, H=16, HS=64).

Sharding (8 cores): core = 4*b + g. Data parallel over batch b in {0,1},
tensor parallel over head-groups g in {0..3} (4 heads / 256 channels each).
Each core computes its heads' attention and a partial output projection
(contracting its 256 channels of w_proj); the host sums the 4 partials per
batch and adds b_proj.

On-device dataflow is fully transposed so no transposes are needed:
  q^T/k^T: [128, T] head-pair tiles (hs on partitions, even head 0:64 /
           odd head 64:128) from lhsT=w_slice, rhs=x^T
  v:       [T, 4*(hs+1)] natural (from lhsT=x^T tile, rhs=w_slice), with
           the weights pre-rearranged on the host so each head carries an
           extra bias=1 column; the PV matmul then also accumulates the
           softmax denominator (row 64 of the accumulator).
  S^T:     [Tk, Tq] = (k @ q^T) blocks; softmax along partitions becomes
           free-axis-independent: exp on ACT, denominator via the ones row.
Causal handling: block (i, j) only computes columns >= 128*(i-4j); the
128-wide diagonal sub-block is masked by ACCUMULATING -30000 into the
masked PSUM entries via one extra matmul (triA^T @ negshift), so exp
underflows them to exactly 0 with no post-exp mask op.
All matmuls run in fp16 (fp32 PSUM accumulation); exp needs no
max-subtraction (scores are ~N(0,1); fp16/fp32 exp range is ample).

Schedule: the attention inner loop is ACT(exp)-throughput-bound, so pure-PE
"filler" units (next chunk's QKV projections, previous chunk's output
projection tiles) are interleaved between attention blocks to keep the PE
busy while ACT catches up.
"""

import numpy as np

import concourse.bass as bass
import concourse.bacc as bacc
import concourse.mybir as mybir
import concourse.tile as tile
from concourse import bass_utils

F16 = mybir.dt.float16
F32 = mybir.dt.float32

B, T, C, H = 2, 2048, 1024, 16
HS = C // H            # 64
G = 4                  # heads per core
CH = G * HS            # 256 channels per core
VC = G * (HS + 1)      # 260 v columns incl. per-head ones column
NK = C // 128          # 8 contraction tiles for the projections
NT = T // 128          # 16 sequence tiles
NCHUNK = T // 512      # 4 Tq chunks

LAST_RESULT = None


def _build_nc(repeat=1):
    nc = bacc.Bacc("TRN2", target_bir_lowering=False)

    xT = nc.dram_tensor("xT", [C, T], F16, kind="ExternalInput")        # x[b].T
    wqk = nc.dram_tensor("wqk", [C, 2 * CH], F16, kind="ExternalInput")  # [C, q|k]
    wv = nc.dram_tensor("wv", [C, VC], F16, kind="ExternalInput")
    wp = nc.dram_tensor("wp", [CH, C], F16, kind="ExternalInput")        # w_proj rows
    bqk = nc.dram_tensor("bqk", [128, 4], F32, kind="ExternalInput")  # col m: 128-ch block m of [b_q|b_k]
    bv = nc.dram_tensor("bv", [128, VC], F32, kind="ExternalInput")   # broadcast rows
    triA = nc.dram_tensor("triA", [128, 128], F16, kind="ExternalInput")   # 1 iff k <= m
    negsh = nc.dram_tensor("negsh", [128, 512], F16, kind="ExternalInput")  # [k,n]=-3e4 iff k==n+1
    out = nc.dram_tensor("out", [T, C], F16, kind="ExternalOutput")

    with tile.TileContext(nc) as tc:
        with (
            tc.tile_pool(name="p_xT", bufs=NK) as p_xT,
            tc.tile_pool(name="p_wqk", bufs=1) as p_wqk,
            tc.tile_pool(name="p_wv", bufs=1) as p_wv,
            tc.tile_pool(name="p_wp", bufs=1) as p_wp,
            tc.tile_pool(name="p_qk", bufs=4) as p_qk,
            tc.tile_pool(name="p_v", bufs=NT) as p_v,
            tc.tile_pool(name="p_pt", bufs=8) as p_pt,
            tc.tile_pool(name="p_yT", bufs=2) as p_yT,
            tc.tile_pool(name="p_out", bufs=6) as p_out,
            tc.tile_pool(name="p_const", bufs=1) as p_const,
            tc.tile_pool(name="p_rc", bufs=8) as p_rc,
            tc.tile_pool(name="p_bc", bufs=8) as p_bc,
            tc.tile_pool(name="ps_mm", bufs=2, space="PSUM") as ps_mm,
            tc.tile_pool(name="ps_st", bufs=2, space="PSUM") as ps_st,
            tc.tile_pool(name="ps_y", bufs=2, space="PSUM") as ps_y,
        ):
            # ---- loads ----
            # single merged DMA per weight tensor (one issue, one transfer);
            # xT issues spread across all three DMA-capable queues so the
            # k-tiles land roughly in consumption order.
            xT_sb = [
                p_xT.tile([128, T], F16, tag="xT", name=f"xT{k}")
                for k in range(NK)
            ]

            def load_xT(k):
                return xT[k * 128:(k + 1) * 128, :]

            wqk_t = p_wqk.tile([128, NK, 2 * CH], F16, tag="wqk")
            # m=0 slice of k=0 first: unblocks the very first matmul sooner
            nc.scalar.dma_start(out=wqk_t[:, 0, 0:128], in_=wqk[0:128, 0:128])
            nc.scalar.dma_start(out=wqk_t[:, 0, 128:], in_=wqk[0:128, 128:])
            wqk_sb = [wqk_t[:, k, :] for k in range(NK)]
            bqk_sb = p_const.tile([128, 4], F32, tag="bqk")
            bv_sb = p_const.tile([128, VC], F32, tag="bv")
            triA_sb = p_const.tile([128, 128], F16, tag="triA")
            negsh_sb = p_const.tile([128, 512], F16, tag="negsh")
            wv_t = p_wv.tile([128, NK, VC], F16, tag="wv")
            wv_sb = [wv_t[:, k, :] for k in range(NK)]
            wp_t = p_wp.tile([128, 2, C], F16, tag="wp")
            wp_sb = [wp_t[:, c, :] for c in range(2)]

            # sync queue; xT k=0 split so its first Tq chunk lands first
            nc.sync.dma_start(out=xT_sb[0][:, 0:512], in_=xT[0:128, 0:512])
            nc.sync.dma_start(out=xT_sb[0][:, 512:], in_=xT[0:128, 512:])
            for k in (1, 3, 4):
                nc.sync.dma_start(out=xT_sb[k], in_=load_xT(k))
            # scalar queue (wqk k=0 already first); wqk staged so each
            # k-group lands just before the k-major QKV loop consumes it
            nc.scalar.dma_start(out=bqk_sb, in_=bqk[:, :])
            nc.scalar.dma_start(
                out=wqk_t[:, 1:4, :],
                in_=wqk[128:512, :].rearrange("(k p) c -> p k c", k=3),
            )
            nc.scalar.dma_start(
                out=wqk_t[:, 4:NK, :],
                in_=wqk[512:, :].rearrange("(k p) c -> p k c", k=NK - 4),
            )
            nc.scalar.dma_start(out=bv_sb, in_=bv[:, :])
            nc.scalar.dma_start(out=triA_sb, in_=triA[:, :])
            nc.scalar.dma_start(out=negsh_sb, in_=negsh[:, :])
            # gpsimd queue (software DGE)
            nc.gpsimd.dma_start(
                out=wv_t, in_=wv.rearrange("(k p) c -> p k c", k=NK)
            )
            for k in (2, 5, 6, 7):
                nc.gpsimd.dma_start(out=xT_sb[k], in_=load_xT(k))
            nc.gpsimd.dma_start(
                out=wp_t, in_=wp.rearrange("(c p) d -> p c d", c=2)
            )

            # q/k head-pair tiles [128, T] fp16: partitions 0:64 = even head,
            # 64:128 = odd head.  0,1 = q pairs; 2,3 = k pairs.
            qk_sb = [
                p_qk.tile([128, T], F16, tag="qk", name=f"qk{i}") for i in range(4)
            ]
            v_sb = [
                p_v.tile([128, G, HS + 1], F16, tag="v", name=f"v{i}")
                for i in range(NT)
            ]
            yT_sb = [
                p_yT.tile([128, T], F16, tag="yT", name=f"yT{i}") for i in range(2)
            ]

            def qk_chunk(m, j):
                """q (m=0,1) / k (m=2,3) projection for Tq chunk j."""
                ps = ps_mm.tile([128, 512], F32, tag="mm", name="ps_qk")
                for k in range(NK):
                    nc.tensor.matmul(
                        ps,
                        lhsT=wqk_sb[k][:, m * 128:(m + 1) * 128],
                        rhs=xT_sb[k][:, j * 512:(j + 1) * 512],
                        start=(k == 0),
                        stop=(k == NK - 1),
                    )
                nc.vector.tensor_scalar_add(
                    out=qk_sb[m][:, j * 512:(j + 1) * 512],
                    in0=ps,
                    scalar1=bqk_sb[:, m:m + 1],
                )

            def v_tile(t):
                """v projection for Tk tile t; bias + per-head ones column
                come from the host-prearranged wv/bv layout."""
                ps = ps_mm.tile([128, 512], F32, tag="mm", name="ps_v")
                for k in range(NK):
                    nc.tensor.matmul(
                        ps[:, 0:VC],
                        lhsT=xT_sb[k][:, t * 128:(t + 1) * 128],
                        rhs=wv_sb[k],
                        start=(k == 0),
                        stop=(k == NK - 1),
                    )
                vt = v_sb[t]
                nc.vector.tensor_add(
                    out=vt[:, :, :],
                    in0=ps[:, 0:VC].rearrange("p (g d) -> p g d", g=G),
                    in1=bv_sb.rearrange("p (g d) -> p g d", g=G),
                )

            def attn_chunk(c, j, popf, tail=False, after_quarter=None):
                """Attention for head pair c (heads 2c, 2c+1), Tq chunk j.
                The two sub-heads' score blocks live in the two banks of one
                [128, 2, 512] PSUM tile (one legal accumulation group per
                bank), sharing a single merged exp on ACT.  popf() emits one
                pure-PE filler unit (next-chunk QKV / prev-chunk proj)."""
                qp = qk_sb[c]
                kp = qk_sb[2 + c]
                nblk = 4 * j + 4
                psy_s = [
                    ps_y.tile([HS + 1, 512], F32, tag="y", name=f"psy{s}")
                    for s in range(2)
                ]

                def st_exp(i):
                    r = i - 4 * j
                    c0 = max(0, r) * 128  # first causally-valid column
                    pss = ps_st.tile([128, 2, 512], F32, tag="st", name="ps_s")
                    for s in range(2):  # sub-head at partitions 64s:64s+64
                        p0 = HS * s
                        nc.tensor.matmul(
                            pss[:, s, c0:512],
                            lhsT=kp[p0:p0 + HS, i * 128:(i + 1) * 128],
                            rhs=qp[p0:p0 + HS, j * 512 + c0:(j + 1) * 512],
                            start=True,
                            stop=(r < 0),
                        )
                        if r >= 0:
                            # accumulate -3e4 into the masked (k > q) entries
                            # of the diagonal sub-block; exp underflows to 0
                            nc.tensor.matmul(
                                pss[:, s, c0:c0 + 128],
                                lhsT=triA_sb,
                                rhs=negsh_sb[:, 0:128],
                                start=False,
                                stop=True,
                                skip_group_check=True,
                            )
                    pt = p_pt.tile([128, 2, 512], F16, tag="pt")
                    nc.scalar.activation(
                        out=pt[:, :, c0:512],
                        in_=pss[:, :, c0:512],
                        func=mybir.ActivationFunctionType.Exp,
                        scale=float(1.0 / np.sqrt(HS)),
                    )
                    return c0, pt

                def pv(i, c0, pt):
                    for s in range(2):
                        nc.tensor.matmul(
                            psy_s[s][:, c0:512],
                            lhsT=v_sb[i][:, 2 * c + s, :],
                            rhs=pt[:, s, c0:512],
                            start=(i == 0),
                            stop=(i == nblk - 1),
                        )

                def norm_quarter(q):
                    """Normalize columns 128q:128(q+1) once their PV
                    accumulation group closed (tail chunk only)."""
                    qlo = 128 * q
                    col = j * 512 + qlo
                    for s in range(2):
                        rc = p_rc.tile([1, 128], F32, tag="rcq")
                        nc.vector.reciprocal(
                            out=rc, in_=psy_s[s][HS:HS + 1, qlo:qlo + 128]
                        )
                        bc = p_bc.tile([HS, 128], F32, tag="bcq")
                        nc.gpsimd.partition_broadcast(bc, rc)
                        nc.vector.tensor_mul(
                            out=yT_sb[c][HS * s:HS * s + HS, col:col + 128],
                            in0=psy_s[s][0:HS, qlo:qlo + 128],
                            in1=bc,
                        )
                    if after_quarter is not None:
                        after_quarter(q)

                def pv_tail(i, c0, pt):
                    """PV for the drain pair: the 4 diagonal blocks close one
                    128-column quarter each, so normalization and the output
                    projection of finished columns pipeline with the
                    remaining attention blocks."""
                    r = i - 4 * j
                    if r < 0:
                        for s in range(2):
                            nc.tensor.matmul(
                                psy_s[s][:, 0:512],
                                lhsT=v_sb[i][:, 2 * c + s, :],
                                rhs=pt[:, s, 0:512],
                                start=(i == 0),
                                stop=False,
                            )
                        return
                    for s in range(2):
                        for q in range(r, 4):
                            qlo = 128 * q
                            nc.tensor.matmul(
                                psy_s[s][:, qlo:qlo + 128],
                                lhsT=v_sb[i][:, 2 * c + s, :],
                                rhs=pt[:, s, qlo:qlo + 128],
                                start=False,
                                stop=(q == r),
                                skip_group_check=True,
                            )
                    norm_quarter(r)

                # one-block software pipeline: ST/exp of block i+1 is queued
                # on PE before PV of block i, so PE never idles on ACT.
                prev = None
                for i in range(nblk):
                    cur = (i, *st_exp(i))
                    if prev is not None:
                        (pv_tail if tail else pv)(*prev)
                    popf()
                    prev = cur
                (pv_tail if tail else pv)(*prev)
                popf()
                if tail:
                    return
                # normalization: 1/denom (row HS) broadcast down the
                # partitions on Pool, multiply straight out of PSUM.
                rcs = []
                for s in range(2):
                    rc = p_rc.tile([1, 512], F32, tag="rc")
                    nc.vector.reciprocal(out=rc, in_=psy_s[s][HS:HS + 1, :])
                    rcs.append(rc)
                bcs = []
                for s in range(2):
                    bc = p_bc.tile([HS, 512], F32, tag="bc")
                    nc.gpsimd.partition_broadcast(bc, rcs[s])
                    bcs.append(bc)
                popf()
                for s in range(2):
                    nc.vector.tensor_mul(
                        out=yT_sb[c][
                            HS * s:HS * s + HS, j * 512:(j + 1) * 512
                        ],
                        in0=psy_s[s][0:HS, :],
                        in1=bcs[s],
                    )

            def proj_tile(t, tail=False, act_copy=False):
                """Output projection for Tq tile t.  Tail tiles skip the SBUF
                staging copy entirely: a gpsimd-issued casting DMA streams the
                PSUM accumulators straight to DRAM (Pool is idle in the
                drain, and this keeps DVE off the critical path)."""
                osb = p_out.tile([128, C], F16, tag="os")
                for o in range(2):
                    ps = ps_mm.tile([128, 512], F32, tag="mm", name="ps_o")
                    for cc in range(2):
                        nc.tensor.matmul(
                            ps,
                            lhsT=yT_sb[cc][:, t * 128:(t + 1) * 128],
                            rhs=wp_sb[cc][:, o * 512:(o + 1) * 512],
                            start=(cc == 0),
                            stop=(cc == 1),
                        )
                    on_act = act_copy == "act" or (act_copy == "split" and o == 1)
                    if on_act:
                        nc.scalar.copy(
                            out=osb[:, o * 512:(o + 1) * 512], in_=ps
                        )
                    else:
                        nc.vector.tensor_copy(
                            out=osb[:, o * 512:(o + 1) * 512], in_=ps
                        )
                if tail:
                    # half-stores issue as soon as each eviction lands
                    nc.sync.dma_start(
                        out=out[t * 128:(t + 1) * 128, 0:512],
                        in_=osb[:, 0:512],
                    )
                    nc.scalar.dma_start(
                        out=out[t * 128:(t + 1) * 128, 512:C],
                        in_=osb[:, 512:C],
                    )
                else:
                    nc.sync.dma_start(
                        out=out[t * 128:(t + 1) * 128, :], in_=osb
                    )

            def qkv0():
                """Chunk-0 QKV, k-major: all four q/k accumulators (and then
                all four v tiles) advance together so each arriving xT k-tile
                unlocks 4 matmuls, hiding the initial DMA latency.  The two
                extra accumulators borrow an idle ps_st tile's banks."""
                ms = (0, 2, 1, 3)
                psa = ps_mm.tile([128, 512], F32, tag="mm", name="ps_qa")
                psb = ps_mm.tile([128, 512], F32, tag="mm", name="ps_qb")
                pscd = ps_st.tile([128, 2, 512], F32, tag="st", name="ps_qcd")
                accs = [psa, psb, pscd[:, 0, :], pscd[:, 1, :]]
                for k in range(NK):
                    for mi, m in enumerate(ms):
                        nc.tensor.matmul(
                            accs[mi],
                            lhsT=wqk_sb[k][:, m * 128:(m + 1) * 128],
                            rhs=xT_sb[k][:, 0:512],
                            start=(k == 0),
                            stop=(k == NK - 1),
                        )
                for mi, m in enumerate(ms):
                    nc.vector.tensor_scalar_add(
                        out=qk_sb[m][:, 0:512],
                        in0=accs[mi],
                        scalar1=bqk_sb[:, m:m + 1],
                    )
                psa2 = ps_mm.tile([128, 512], F32, tag="mm", name="ps_va")
                psb2 = ps_mm.tile([128, 512], F32, tag="mm", name="ps_vb")
                pscd2 = ps_st.tile([128, 2, 512], F32, tag="st", name="ps_vcd")
                vaccs = [
                    psa2[:, 0:VC],
                    psb2[:, 0:VC],
                    pscd2[:, 0, 0:VC],
                    pscd2[:, 1, 0:VC],
                ]
                for k in range(NK):
                    for t in range(4):
                        nc.tensor.matmul(
                            vaccs[t],
                            lhsT=xT_sb[k][:, t * 128:(t + 1) * 128],
                            rhs=wv_sb[k],
                            start=(k == 0),
                            stop=(k == NK - 1),
                        )
                for t in range(4):
                    nc.vector.tensor_add(
                        out=v_sb[t][:, :, :],
                        in0=vaccs[t].rearrange("p (g d) -> p g d", g=G),
                        in1=bv_sb.rearrange("p (g d) -> p g d", g=G),
                    )

            # ---- schedule ----
            # chunk 0's QKV up front; then per chunk j: both attention pairs
            # with next-chunk QKV + prev-chunk proj interleaved as fillers;
            # the final chunk's proj drains at the end.
            def body():
                qkv0()
                for j in range(NCHUNK):
                    # urgent units are emitted one-per-pop before the evenly
                    # spread ones; chunk 3's own v tiles must land before
                    # their PV consumers in the same chunk.
                    urgent = []
                    units = []
                    if j + 1 < NCHUNK:
                        units += [
                            lambda m=m, j1=j + 1: qk_chunk(m, j1)
                            for m in (0, 2, 1, 3)
                        ]
                        if j + 1 < NCHUNK - 1:
                            units += [
                                lambda t=t: v_tile(t)
                                for t in range(4 * j + 4, 4 * j + 8)
                            ]
                        else:  # last chunk's v tiles become its own fillers
                            units += [
                                lambda t=t: v_tile(t)
                                for t in range(4 * j + 4, 4 * j + 6)
                            ]
                            urgent_next = [
                                lambda t=t: v_tile(t)
                                for t in range(4 * j + 6, 4 * j + 8)
                            ]
                    if j == NCHUNK - 1:
                        urgent += urgent_next
                    if j > 0:
                        proj_us = [
                            lambda t=t: proj_tile(t)
                            for t in range(4 * (j - 1), 4 * j)
                        ]
                        if j == NCHUNK - 2:
                            # defer two proj tiles into the last chunk, whose
                            # ACT-paced stretch is the longest on hardware
                            units += proj_us[:2]
                            deferred_proj = proj_us[2:]
                        elif j == NCHUNK - 1:
                            units = deferred_proj + units + proj_us
                        else:
                            units += proj_us
                    # pop points: one per block plus 2 extra per pair
                    npop = 2 * (4 * j + 4) + 4
                    emitted = [0]
                    calls = [0]
                    nunits = len(units)

                    def popf():
                        if urgent:
                            urgent.pop(0)()
                            return
                        calls[0] += 1
                        while units and emitted[0] * npop < calls[0] * nunits:
                            u = units.pop(0)
                            u()
                            emitted[0] += 1

                    last = j == NCHUNK - 1
                    attn_chunk(0, j, popf)
                    attn_chunk(
                        1,
                        j,
                        popf,
                        tail=last,
                        after_quarter=(
                            (
                                lambda q: proj_tile(
                                    4 * j + q,
                                    tail=True,
                                    act_copy=(
                                        "split"
                                        if q == 3
                                        else ("act" if q == 2 else False)
                                    ),
                                )
                            )
                            if last
                            else None
                        ),
                    )
                    while urgent:
                        urgent.pop(0)()
                    while units:
                        units.pop(0)()

            if repeat == 1:
                body()
            else:  # benchmarking only: loop the whole compute on-device
                with tc.For_i(0, repeat, 1):
                    body()

    nc.finalize()
    return nc


_NC = None


def _get_nc():
    global _NC
    if _NC is None:
        _NC = _build_nc()
    return _NC


def _make_in_maps(x, w_attn, b_attn, w_proj):
    triA = (np.arange(128)[:, None] <= np.arange(128)[None, :]).astype(np.float16)
    negsh = np.zeros((128, 512), dtype=np.float16)
    negsh[np.arange(1, 128), np.arange(127)] = -30000.0
    in_maps = []
    for core in range(8):
        b, g = divmod(core, 4)
        c0 = CH * g
        xTb = np.ascontiguousarray(x[b].T).astype(np.float16)
        wqk = np.concatenate(
            [w_attn[:, c0:c0 + CH], w_attn[:, C + c0:C + c0 + CH]], axis=1
        ).astype(np.float16)
        # v weights/bias with a zero/one ones-column per head
        wv_flat = w_attn[:, 2 * C + c0:2 * C + c0 + CH]  # [C, CH]
        wv = np.zeros((C, VC), dtype=np.float16)
        bv = np.zeros((128, VC), dtype=np.float32)
        bvec = b_attn[2 * C + c0:2 * C + c0 + CH]
        for gg in range(G):
            wv[:, gg * (HS + 1):gg * (HS + 1) + HS] = wv_flat[
                :, gg * HS:(gg + 1) * HS
            ]
            bv[:, gg * (HS + 1):gg * (HS + 1) + HS] = bvec[
                None, gg * HS:(gg + 1) * HS
            ]
            bv[:, gg * (HS + 1) + HS] = 1.0
        wp = np.ascontiguousarray(w_proj[c0:c0 + CH, :]).astype(np.float16)
        bqk = np.concatenate(
            [b_attn[c0:c0 + CH], b_attn[C + c0:C + c0 + CH]]
        ).reshape(4, 128).T.astype(np.float32)
        bqk = np.ascontiguousarray(bqk)
        in_maps.append(
            {
                "xT": xTb,
                "wqk": wqk,
                "wv": wv,
                "wp": wp,
                "bqk": bqk,
                "bv": bv,
                "triA": triA,
                "negsh": negsh,
            }
        )
    return in_maps


def kernel(x, w_attn, b_attn, w_proj, b_proj, trace=False):
    global LAST_RESULT
    x = np.asarray(x, dtype=np.float32)
    w_attn = np.asarray(w_attn, dtype=np.float32)
    b_attn = np.asarray(b_attn, dtype=np.float32)
    w_proj = np.asarray(w_proj, dtype=np.float32)
    b_proj = np.asarray(b_proj, dtype=np.float32)

    nc = _get_nc()
    in_maps = _make_in_maps(x, w_attn, b_attn, w_proj)
    res = bass_utils.run_bass_kernel_spmd(
        nc, in_maps, core_ids=list(range(8)), trace=trace
    )
    LAST_RESULT = res
    parts = [r["out"] for r in res.results]
    out = np.empty((B, T, C), dtype=np.float32)
    for b in range(B):
        acc = parts[4 * b].astype(np.float32)
        for g in range(1, 4):
            acc = acc + parts[4 * b + g].astype(np.float32)
        out[b] = acc + b_proj[None, :]
    return out


# revision 48
# speedup vs baseline: 1.0624x; 1.0624x over previous
"""CausalSelfAttention Trainium2 kernel (B=2, T=2048, C=1024, H=16, HS=64).

Sharding (8 cores): core = 4*b + g. Data parallel over batch b in {0,1},
tensor parallel over head-groups g in {0..3} (4 heads / 256 channels each).
Each core computes its heads' attention and a partial output projection
(contracting its 256 channels of w_proj); the host sums the 4 partials per
batch and adds b_proj.

On-device dataflow is fully transposed so no transposes are needed:
  q^T/k^T: [128, T] head-pair tiles (hs on partitions, even head 0:64 /
           odd head 64:128) from lhsT=w_slice, rhs=x^T
  v:       [T, 4*(hs+1)] natural (from lhsT=x^T tile, rhs=w_slice), with
           the weights pre-rearranged on the host so each head carries an
           extra bias=1 column; the PV matmul then also accumulates the
           softmax denominator (row 64 of the accumulator).
  S^T:     [Tk, Tq] = (k @ q^T) blocks; softmax along partitions becomes
           free-axis-independent: exp on ACT, denominator via the ones row.
Causal handling: block (i, j) only computes columns >= 128*(i-4j); the
128-wide diagonal sub-block is masked by ACCUMULATING -30000 into the
masked PSUM entries via one extra matmul (triA^T @ negshift), so exp
underflows them to exactly 0 with no post-exp mask op.
All matmuls run in fp16 (fp32 PSUM accumulation); exp needs no
max-subtraction (scores are ~N(0,1); fp16/fp32 exp range is ample).

Schedule: the attention inner loop is ACT(exp)-throughput-bound, so pure-PE
"filler" units (next chunk's QKV projections, previous chunk's output
projection tiles) are interleaved between attention blocks to keep the PE
busy while ACT catches up.
"""

import numpy as np

import concourse.bass as bass
import concourse.bacc as bacc
import concourse.mybir as mybir
import concourse.tile as tile
from concourse import bass_utils

F16 = mybir.dt.float16
F32 = mybir.dt.float32

B, T, C, H = 2, 2048, 1024, 16
HS = C // H            # 64
G = 4                  # heads per core
CH = G * HS            # 256 channels per core
VC = G * (HS + 1)      # 260 v columns incl. per-head ones column
NK = C // 128          # 8 contraction tiles for the projections
NT = T // 128          # 16 sequence tiles
NCHUNK = T // 512      # 4 Tq chunks

LAST_RESULT = None


def _build_nc(repeat=1):
    nc = bacc.Bacc("TRN2", target_bir_lowering=False)

    xT = nc.dram_tensor("xT", [C, T], F16, kind="ExternalInput")        # x[b].T
    wqk = nc.dram_tensor("wqk", [C, 2 * CH], F16, kind="ExternalInput")  # [C, q|k]
    wv = nc.dram_tensor("wv", [C, VC], F16, kind="ExternalInput")
    wp = nc.dram_tensor("wp", [CH, C], F16, kind="ExternalInput")        # w_proj rows
    bqk = nc.dram_tensor("bqk", [128, 4], F32, kind="ExternalInput")  # col m: 128-ch block m of [b_q|b_k]
    bv = nc.dram_tensor("bv", [128, VC], F32, kind="ExternalInput")   # broadcast rows
    triA = nc.dram_tensor("triA", [128, 128], F16, kind="ExternalInput")   # 1 iff k <= m
    negsh = nc.dram_tensor("negsh", [128, 512], F16, kind="ExternalInput")  # [k,n]=-3e4 iff k==n+1
    out = nc.dram_tensor("out", [T, C], F16, kind="ExternalOutput")

    with tile.TileContext(nc) as tc:
        with (
            tc.tile_pool(name="p_xT", bufs=NK) as p_xT,
            tc.tile_pool(name="p_wqk", bufs=1) as p_wqk,
            tc.tile_pool(name="p_wv", bufs=1) as p_wv,
            tc.tile_pool(name="p_wp", bufs=1) as p_wp,
            tc.tile_pool(name="p_qk", bufs=4) as p_qk,
            tc.tile_pool(name="p_v", bufs=NT) as p_v,
            tc.tile_pool(name="p_pt", bufs=8) as p_pt,
            tc.tile_pool(name="p_yT", bufs=2) as p_yT,
            tc.tile_pool(name="p_out", bufs=6) as p_out,
            tc.tile_pool(name="p_const", bufs=1) as p_const,
            tc.tile_pool(name="p_rc", bufs=8) as p_rc,
            tc.tile_pool(name="p_bc", bufs=8) as p_bc,
            tc.tile_pool(name="ps_mm", bufs=2, space="PSUM") as ps_mm,
            tc.tile_pool(name="ps_st", bufs=2, space="PSUM") as ps_st,
            tc.tile_pool(name="ps_y", bufs=2, space="PSUM") as ps_y,
        ):
            # ---- loads ----
            # single merged DMA per weight tensor (one issue, one transfer);
            # xT issues spread across all three DMA-capable queues so the
            # k-tiles land roughly in consumption order.
            xT_sb = [
                p_xT.tile([128, T], F16, tag="xT", name=f"xT{k}")
                for k in range(NK)
            ]

            def load_xT(k):
                return xT[k * 128:(k + 1) * 128, :]

            wqk_t = p_wqk.tile([128, NK, 2 * CH], F16, tag="wqk")
            # m=0 slice of k=0 first: unblocks the very first matmul sooner
            nc.scalar.dma_start(out=wqk_t[:, 0, 0:128], in_=wqk[0:128, 0:128])
            nc.scalar.dma_start(out=wqk_t[:, 0, 128:], in_=wqk[0:128, 128:])
            wqk_sb = [wqk_t[:, k, :] for k in range(NK)]
            bqk_sb = p_const.tile([128, 4], F32, tag="bqk")
            bv_sb = p_const.tile([128, VC], F32, tag="bv")
            triA_sb = p_const.tile([128, 128], F16, tag="triA")
            negsh_sb = p_const.tile([128, 512], F16, tag="negsh")
            wv_t = p_wv.tile([128, NK, VC], F16, tag="wv")
            wv_sb = [wv_t[:, k, :] for k in range(NK)]
            wp_t = p_wp.tile([128, 2, C], F16, tag="wp")
            wp_sb = [wp_t[:, c, :] for c in range(2)]

            # sync queue; xT k=0 split so its first Tq chunk lands first
            nc.sync.dma_start(out=xT_sb[0][:, 0:512], in_=xT[0:128, 0:512])
            nc.sync.dma_start(out=xT_sb[0][:, 512:], in_=xT[0:128, 512:])
            for k in (1, 3, 4):
                nc.sync.dma_start(out=xT_sb[k], in_=load_xT(k))
            # scalar queue (wqk k=0 already first); wqk staged so each
            # k-group lands just before the k-major QKV loop consumes it
            nc.scalar.dma_start(out=bqk_sb, in_=bqk[:, :])
            nc.scalar.dma_start(
                out=wqk_t[:, 1:4, :],
                in_=wqk[128:512, :].rearrange("(k p) c -> p k c", k=3),
            )
            nc.scalar.dma_start(
                out=wqk_t[:, 4:NK, :],
                in_=wqk[512:, :].rearrange("(k p) c -> p k c", k=NK - 4),
            )
            nc.scalar.dma_start(out=bv_sb, in_=bv[:, :])
            nc.scalar.dma_start(out=triA_sb, in_=triA[:, :])
            nc.scalar.dma_start(out=negsh_sb, in_=negsh[:, :])
            # gpsimd queue (software DGE)
            nc.gpsimd.dma_start(
                out=wv_t, in_=wv.rearrange("(k p) c -> p k c", k=NK)
            )
            for k in (2, 5, 6, 7):
                nc.gpsimd.dma_start(out=xT_sb[k], in_=load_xT(k))
            nc.gpsimd.dma_start(
                out=wp_t, in_=wp.rearrange("(c p) d -> p c d", c=2)
            )

            # q/k head-pair tiles [128, T] fp16: partitions 0:64 = even head,
            # 64:128 = odd head.  0,1 = q pairs; 2,3 = k pairs.
            qk_sb = [
                p_qk.tile([128, T], F16, tag="qk", name=f"qk{i}") for i in range(4)
            ]
            v_sb = [
                p_v.tile([128, G, HS + 1], F16, tag="v", name=f"v{i}")
                for i in range(NT)
            ]
            yT_sb = [
                p_yT.tile([128, T], F16, tag="yT", name=f"yT{i}") for i in range(2)
            ]

            def qk_chunk(m, j):
                """q (m=0,1) / k (m=2,3) projection for Tq chunk j."""
                ps = ps_mm.tile([128, 512], F32, tag="mm", name="ps_qk")
                for k in range(NK):
                    nc.tensor.matmul(
                        ps,
                        lhsT=wqk_sb[k][:, m * 128:(m + 1) * 128],
                        rhs=xT_sb[k][:, j * 512:(j + 1) * 512],
                        start=(k == 0),
                        stop=(k == NK - 1),
                    )
                nc.vector.tensor_scalar_add(
                    out=qk_sb[m][:, j * 512:(j + 1) * 512],
                    in0=ps,
                    scalar1=bqk_sb[:, m:m + 1],
                )

            def v_tile(t):
                """v projection for Tk tile t; bias + per-head ones column
                come from the host-prearranged wv/bv layout."""
                ps = ps_mm.tile([128, 512], F32, tag="mm", name="ps_v")
                for k in range(NK):
                    nc.tensor.matmul(
                        ps[:, 0:VC],
                        lhsT=xT_sb[k][:, t * 128:(t + 1) * 128],
                        rhs=wv_sb[k],
                        start=(k == 0),
                        stop=(k == NK - 1),
                    )
                vt = v_sb[t]
                nc.vector.tensor_add(
                    out=vt[:, :, :],
                    in0=ps[:, 0:VC].rearrange("p (g d) -> p g d", g=G),
                    in1=bv_sb.rearrange("p (g d) -> p g d", g=G),
                )

            def attn_chunk(c, j, popf, tail=False, after_quarter=None):
                """Attention for head pair c (heads 2c, 2c+1), Tq chunk j.
                The two sub-heads' score blocks live in the two banks of one
                [128, 2, 512] PSUM tile (one legal accumulation group per
                bank), sharing a single merged exp on ACT.  popf() emits one
                pure-PE filler unit (next-chunk QKV / prev-chunk proj)."""
                qp = qk_sb[c]
                kp = qk_sb[2 + c]
                nblk = 4 * j + 4
                psy_s = [
                    ps_y.tile([HS + 1, 512], F32, tag="y", name=f"psy{s}")
                    for s in range(2)
                ]

                def st_exp(i):
                    r = i - 4 * j
                    c0 = max(0, r) * 128  # first causally-valid column
                    pss = ps_st.tile([128, 2, 512], F32, tag="st", name="ps_s")
                    for s in range(2):  # sub-head at partitions 64s:64s+64
                        p0 = HS * s
                        nc.tensor.matmul(
                            pss[:, s, c0:512],
                            lhsT=kp[p0:p0 + HS, i * 128:(i + 1) * 128],
                            rhs=qp[p0:p0 + HS, j * 512 + c0:(j + 1) * 512],
                            start=True,
                            stop=(r < 0),
                        )
                        if r >= 0:
                            # accumulate -3e4 into the masked (k > q) entries
                            # of the diagonal sub-block; exp underflows to 0
                            nc.tensor.matmul(
                                pss[:, s, c0:c0 + 128],
                                lhsT=triA_sb,
                                rhs=negsh_sb[:, 0:128],
                                start=False,
                                stop=True,
                                skip_group_check=True,
                            )
                    pt = p_pt.tile([128, 2, 512], F16, tag="pt")
                    nc.scalar.activation(
                        out=pt[:, :, c0:512],
                        in_=pss[:, :, c0:512],
                        func=mybir.ActivationFunctionType.Exp,
                        scale=float(1.0 / np.sqrt(HS)),
                    )
                    return c0, pt

                def pv(i, c0, pt):
                    for s in range(2):
                        nc.tensor.matmul(
                            psy_s[s][:, c0:512],
                            lhsT=v_sb[i][:, 2 * c + s, :],
                            rhs=pt[:, s, c0:512],
                            start=(i == 0),
                            stop=(i == nblk - 1),
                        )

                def norm_quarter(q):
                    """Normalize columns 128q:128(q+1) once their PV
                    accumulation group closed (tail chunk only)."""
                    qlo = 128 * q
                    col = j * 512 + qlo
                    for s in range(2):
                        rc = p_rc.tile([1, 128], F32, tag="rcq")
                        nc.vector.reciprocal(
                            out=rc, in_=psy_s[s][HS:HS + 1, qlo:qlo + 128]
                        )
                        bc = p_bc.tile([HS, 128], F32, tag="bcq")
                        nc.gpsimd.partition_broadcast(bc, rc)
                        nc.vector.tensor_mul(
                            out=yT_sb[c][HS * s:HS * s + HS, col:col + 128],
                            in0=psy_s[s][0:HS, qlo:qlo + 128],
                            in1=bc,
                        )
                    if after_quarter is not None:
                        after_quarter(q)

                def pv_tail(i, c0, pt):
                    """PV for the drain pair: the 4 diagonal blocks close one
                    128-column quarter each, so normalization and the output
                    projection of finished columns pipeline with the
                    remaining attention blocks."""
                    r = i - 4 * j
                    if r < 0:
                        for s in range(2):
                            nc.tensor.matmul(
                                psy_s[s][:, 0:512],
                                lhsT=v_sb[i][:, 2 * c + s, :],
                                rhs=pt[:, s, 0:512],
                                start=(i == 0),
                                stop=False,
                            )
                        return
                    for s in range(2):
                        for q in range(r, 4):
                            qlo = 128 * q
                            nc.tensor.matmul(
                                psy_s[s][:, qlo:qlo + 128],
                                lhsT=v_sb[i][:, 2 * c + s, :],
                                rhs=pt[:, s, qlo:qlo + 128],
                                start=False,
                                stop=(q == r),
                                skip_group_check=True,
                            )
                    norm_quarter(r)

                # one-block software pipeline: ST/exp of block i+1 is queued
                # on PE before PV of block i, so PE never idles on ACT.
                prev = None
                for i in range(nblk):
                    cur = (i, *st_exp(i))
                    if prev is not None:
                        (pv_tail if tail else pv)(*prev)
                    popf()
                    prev = cur
                (pv_tail if tail else pv)(*prev)
                popf()
                if tail:
                    return
                # normalization: 1/denom (row HS) broadcast down the
                # partitions on Pool, multiply straight out of PSUM.
                rcs = []
                for s in range(2):
                    rc = p_rc.tile([1, 512], F32, tag="rc")
                    nc.vector.reciprocal(out=rc, in_=psy_s[s][HS:HS + 1, :])
                    rcs.append(rc)
                bcs = []
                for s in range(2):
                    bc = p_bc.tile([HS, 512], F32, tag="bc")
                    nc.gpsimd.partition_broadcast(bc, rcs[s])
                    bcs.append(bc)
                popf()
                for s in range(2):
                    nc.vector.tensor_mul(
                        out=yT_sb[c][
                            HS * s:HS * s + HS, j * 512:(j + 1) * 512
                        ],
                        in0=psy_s[s][0:HS, :],
                        in1=bcs[s],
                    )

            def proj_tile(t, tail=False, act_copy=False):
                """Output projection for Tq tile t.  Tail tiles skip the SBUF
                staging copy entirely: a gpsimd-issued casting DMA streams the
                PSUM accumulators straight to DRAM (Pool is idle in the
                drain, and this keeps DVE off the critical path)."""
                osb = p_out.tile([128, C], F16, tag="os")
                for o in range(2):
                    ps = ps_mm.tile([128, 512], F32, tag="mm", name="ps_o")
                    for cc in range(2):
                        nc.tensor.matmul(
                            ps,
                            lhsT=yT_sb[cc][:, t * 128:(t + 1) * 128],
                            rhs=wp_sb[cc][:, o * 512:(o + 1) * 512],
                            start=(cc == 0),
                            stop=(cc == 1),
                        )
                    on_act = act_copy == "act" or (act_copy == "split" and o == 1)
                    if on_act:
                        nc.scalar.copy(
                            out=osb[:, o * 512:(o + 1) * 512], in_=ps
                        )
                    else:
                        nc.vector.tensor_copy(
                            out=osb[:, o * 512:(o + 1) * 512], in_=ps
                        )
                if tail:
                    # half-stores issue as soon as each eviction lands
                    nc.sync.dma_start(
                        out=out[t * 128:(t + 1) * 128, 0:512],
                        in_=osb[:, 0:512],
                    )
                    nc.scalar.dma_start(
                        out=out[t * 128:(t + 1) * 128, 512:C],
                        in_=osb[:, 512:C],
                    )
                else:
                    nc.sync.dma_start(
                        out=out[t * 128:(t + 1) * 128, :], in_=osb
                    )

            def qkv0():
                """Chunk-0 QKV, k-major: all four q/k accumulators (and then
                all four v tiles) advance together so each arriving xT k-tile
                unlocks 4 matmuls, hiding the initial DMA latency.  The two
                extra accumulators borrow an idle ps_st tile's banks."""
                ms = (0, 2, 1, 3)
                psa = ps_mm.tile([128, 512], F32, tag="mm", name="ps_qa")
                psb = ps_mm.tile([128, 512], F32, tag="mm", name="ps_qb")
                pscd = ps_st.tile([128, 2, 512], F32, tag="st", name="ps_qcd")
                accs = [psa, psb, pscd[:, 0, :], pscd[:, 1, :]]
                for k in range(NK):
                    for mi, m in enumerate(ms):
                        nc.tensor.matmul(
                            accs[mi],
                            lhsT=wqk_sb[k][:, m * 128:(m + 1) * 128],
                            rhs=xT_sb[k][:, 0:512],
                            start=(k == 0),
                            stop=(k == NK - 1),
                        )
                for mi, m in enumerate(ms):
                    nc.vector.tensor_scalar_add(
                        out=qk_sb[m][:, 0:512],
                        in0=accs[mi],
                        scalar1=bqk_sb[:, m:m + 1],
                    )
                psa2 = ps_mm.tile([128, 512], F32, tag="mm", name="ps_va")
                psb2 = ps_mm.tile([128, 512], F32, tag="mm", name="ps_vb")
                pscd2 = ps_st.tile([128, 2, 512], F32, tag="st", name="ps_vcd")
                vaccs = [
                    psa2[:, 0:VC],
                    psb2[:, 0:VC],
                    pscd2[:, 0, 0:VC],
                    pscd2[:, 1, 0:VC],
                ]
                for k in range(NK):
                    for t in range(4):
                        nc.tensor.matmul(
                            vaccs[t],
                            lhsT=xT_sb[k][:, t * 128:(t + 1) * 128],
                            rhs=wv_sb[k],
                            start=(k == 0),
                            stop=(k == NK - 1),
                        )
                for t in range(4):
                    nc.vector.tensor_add(
                        out=v_sb[t][:, :, :],
                        in0=vaccs[t].rearrange("p (g d) -> p g d", g=G),
                        in1=bv_sb.rearrange("p (g d) -> p g d", g=G),
                    )

            # ---- schedule ----
            # chunk 0's QKV up front; then per chunk j: both attention pairs
            # with next-chunk QKV + prev-chunk proj interleaved as fillers;
            # the final chunk's proj drains at the end.
            def body():
                qkv0()
                for j in range(NCHUNK):
                    # urgent units are emitted one-per-pop before the evenly
                    # spread ones; chunk 3's own v tiles must land before
                    # their PV consumers in the same chunk.
                    urgent = []
                    units = []
                    if j + 1 < NCHUNK:
                        units += [
                            lambda m=m, j1=j + 1: qk_chunk(m, j1)
                            for m in (0, 2, 1, 3)
                        ]
                        if j + 1 < NCHUNK - 1:
                            units += [
                                lambda t=t: v_tile(t)
                                for t in range(4 * j + 4, 4 * j + 8)
                            ]
                        else:  # last chunk's v tiles become its own fillers
                            units += [
                                lambda t=t: v_tile(t)
                                for t in range(4 * j + 4, 4 * j + 6)
                            ]
                            urgent_next = [
                                lambda t=t: v_tile(t)
                                for t in range(4 * j + 6, 4 * j + 8)
                            ]
                    if j == NCHUNK - 1:
                        urgent += urgent_next
                    if j > 0:
                        units += [
                            lambda t=t: proj_tile(t)
                            for t in range(4 * (j - 1), 4 * j)
                        ]
                    # pop points: one per block plus 2 extra per pair
                    npop = 2 * (4 * j + 4) + 4
                    emitted = [0]
                    calls = [0]
                    nunits = len(units)

                    def popf():
                        if urgent:
                            urgent.pop(0)()
                            return
                        calls[0] += 1
                        while units and emitted[0] * npop < calls[0] * nunits:
                            u = units.pop(0)
                            u()
                            emitted[0] += 1

                    last = j == NCHUNK - 1
                    attn_chunk(0, j, popf)
                    attn_chunk(
                        1,
                        j,
                        popf,
                        tail=last,
                        after_quarter=(
                            (
                                lambda q: proj_tile(
                                    4 * j + q,
                                    tail=True,
                                    act_copy=(
                                        "split"
                                        if q == 3
                                        else ("act" if q == 2 else False)
                                    ),
                                )
                            )
                            if last
                            else None
                        ),
                    )
                    while urgent:
                        urgent.pop(0)()
                    while units:
                        units.pop(0)()

            if repeat == 1:
                body()
            else:  # benchmarking only: loop the whole compute on-device
                with tc.For_i(0, repeat, 1):
                    body()

    nc.finalize()
    return nc


_NC = None


def _get_nc():
    global _NC
    if _NC is None:
        _NC = _build_nc()
    return _NC


def _make_in_maps(x, w_attn, b_attn, w_proj):
    triA = (np.arange(128)[:, None] <= np.arange(128)[None, :]).astype(np.float16)
    negsh = np.zeros((128, 512), dtype=np.float16)
    negsh[np.arange(1, 128), np.arange(127)] = -30000.0
    in_maps = []
    for core in range(8):
        b, g = divmod(core, 4)
        c0 = CH * g
        xTb = np.ascontiguousarray(x[b].T).astype(np.float16)
        wqk = np.concatenate(
            [w_attn[:, c0:c0 + CH], w_attn[:, C + c0:C + c0 + CH]], axis=1
        ).astype(np.float16)
        # v weights/bias with a zero/one ones-column per head
        wv_flat = w_attn[:, 2 * C + c0:2 * C + c0 + CH]  # [C, CH]
        wv = np.zeros((C, VC), dtype=np.float16)
        bv = np.zeros((128, VC), dtype=np.float32)
        bvec = b_attn[2 * C + c0:2 * C + c0 + CH]
        for gg in range(G):
            wv[:, gg * (HS + 1):gg * (HS + 1) + HS] = wv_flat[
                :, gg * HS:(gg + 1) * HS
            ]
            bv[:, gg * (HS + 1):gg * (HS + 1) + HS] = bvec[
                None, gg * HS:(gg + 1) * HS
            ]
            bv[:, gg * (HS + 1) + HS] = 1.0
        wp = np.ascontiguousarray(w_proj[c0:c0 + CH, :]).astype(np.float16)
        bqk = np.concatenate(
            [b_attn[c0:c0 + CH], b_attn[C + c0:C + c0 + CH]]
        ).reshape(4, 128).T.astype(np.float32)
        bqk = np.ascontiguousarray(bqk)
        in_maps.append(
            {
                "xT": xTb,
                "wqk": wqk,
                "wv": wv,
                "wp": wp,
                "bqk": bqk,
                "bv": bv,
                "triA": triA,
                "negsh": negsh,
            }
        )
    return in_maps


def kernel(x, w_attn, b_attn, w_proj, b_proj, trace=False):
    global LAST_RESULT
    x = np.asarray(x, dtype=np.float32)
    w_attn = np.asarray(w_attn, dtype=np.float32)
    b_attn = np.asarray(b_attn, dtype=np.float32)
    w_proj = np.asarray(w_proj, dtype=np.float32)
    b_proj = np.asarray(b_proj, dtype=np.float32)

    nc = _get_nc()
    in_maps = _make_in_maps(x, w_attn, b_attn, w_proj)
    res = bass_utils.run_bass_kernel_spmd(
        nc, in_maps, core_ids=list(range(8)), trace=trace
    )
    LAST_RESULT = res
    parts = [r["out"] for r in res.results]
    out = np.empty((B, T, C), dtype=np.float32)
    for b in range(B):
        acc = parts[4 * b].astype(np.float32)
        for g in range(1, 4):
            acc = acc + parts[4 * b + g].astype(np.float32)
        out[b] = acc + b_proj[None, :]
    return out


# revision 54
# speedup vs baseline: 1.2031x; 1.1324x over previous
"""CausalSelfAttention Trainium2 kernel (B=2, T=2048, C=1024, H=16, HS=64).

Sharding (8 cores): core = 4*b + g. Data parallel over batch b in {0,1},
tensor parallel over head-groups g in {0..3} (4 heads / 256 channels each).
Each core computes its heads' attention and a partial output projection
(contracting its 256 channels of w_proj); the host sums the 4 partials per
batch and adds b_proj.

On-device dataflow is fully transposed so no transposes are needed:
  q^T/k^T: [128, T] head-pair tiles (hs on partitions, even head 0:64 /
           odd head 64:128) from lhsT=w_slice, rhs=x^T
  v:       [T, 4*(hs+1)] natural (from lhsT=x^T tile, rhs=w_slice), with
           the weights pre-rearranged on the host so each head carries an
           extra bias=1 column; the PV matmul then also accumulates the
           softmax denominator (row 64 of the accumulator).
  S^T:     [Tk, Tq] = (k @ q^T) blocks; softmax along partitions becomes
           free-axis-independent: exp on ACT, denominator via the ones row.
Causal handling: block (i, j) only computes columns >= 128*(i-4j); the
128-wide diagonal sub-block is masked by ACCUMULATING -30000 into the
masked PSUM entries via one extra matmul (triA^T @ negshift), so exp
underflows them to exactly 0 with no post-exp mask op.
All matmuls run in fp16 (fp32 PSUM accumulation); exp needs no
max-subtraction (scores are ~N(0,1); fp16/fp32 exp range is ample).

Schedule: the attention inner loop is ACT(exp)-throughput-bound, so pure-PE
"filler" units (next chunk's QKV projections, previous chunk's output
projection tiles) are interleaved between attention blocks to keep the PE
busy while ACT catches up.
"""

import numpy as np

import concourse.bass as bass
import concourse.bacc as bacc
import concourse.mybir as mybir
import concourse.tile as tile
from concourse import bass_utils

F16 = mybir.dt.float16
F32 = mybir.dt.float32

B, T, C, H = 2, 2048, 1024, 16
HS = C // H            # 64
G = 4                  # heads per core
CH = G * HS            # 256 channels per core
VC = G * (HS + 1)      # 260 v columns incl. per-head ones column
NK = C // 128          # 8 contraction tiles for the projections
NT = T // 128          # 16 sequence tiles
NCHUNK = T // 512      # 4 Tq chunks

LAST_RESULT = None


def _build_nc(repeat=1):
    nc = bacc.Bacc("TRN2", target_bir_lowering=False)

    xT = nc.dram_tensor("xT", [C, T], F16, kind="ExternalInput")        # x[b].T
    wqk = nc.dram_tensor("wqk", [C, 2 * CH], F16, kind="ExternalInput")  # [C, q|k]
    wv = nc.dram_tensor("wv", [C, VC], F16, kind="ExternalInput")
    wp = nc.dram_tensor("wp", [CH, C], F16, kind="ExternalInput")        # w_proj rows
    bqk = nc.dram_tensor("bqk", [128, 4], F32, kind="ExternalInput")  # col m: 128-ch block m of [b_q|b_k]
    bv = nc.dram_tensor("bv", [128, VC], F32, kind="ExternalInput")   # broadcast rows
    tri2 = nc.dram_tensor("tri2", [128, 2, 128], F16, kind="ExternalInput")  # 1 iff k <= q, both banks
    out = nc.dram_tensor("out", [T, C], F16, kind="ExternalOutput")

    with tile.TileContext(nc) as tc:
        with (
            tc.tile_pool(name="p_xT", bufs=NK) as p_xT,
            tc.tile_pool(name="p_wqk", bufs=1) as p_wqk,
            tc.tile_pool(name="p_wv", bufs=1) as p_wv,
            tc.tile_pool(name="p_wp", bufs=1) as p_wp,
            tc.tile_pool(name="p_qk", bufs=4) as p_qk,
            tc.tile_pool(name="p_v", bufs=NT) as p_v,
            tc.tile_pool(name="p_pt", bufs=8) as p_pt,
            tc.tile_pool(name="p_yT", bufs=2) as p_yT,
            tc.tile_pool(name="p_out", bufs=6) as p_out,
            tc.tile_pool(name="p_const", bufs=1) as p_const,
            tc.tile_pool(name="p_rc", bufs=8) as p_rc,
            tc.tile_pool(name="p_bc", bufs=8) as p_bc,
            tc.tile_pool(name="ps_mm", bufs=2, space="PSUM") as ps_mm,
            tc.tile_pool(name="ps_st", bufs=2, space="PSUM") as ps_st,
            tc.tile_pool(name="ps_y", bufs=2, space="PSUM") as ps_y,
        ):
            # ---- loads ----
            # single merged DMA per weight tensor (one issue, one transfer);
            # xT issues spread across all three DMA-capable queues so the
            # k-tiles land roughly in consumption order.
            xT_sb = [
                p_xT.tile([128, T], F16, tag="xT", name=f"xT{k}")
                for k in range(NK)
            ]

            def load_xT(k):
                return xT[k * 128:(k + 1) * 128, :]

            wqk_t = p_wqk.tile([128, NK, 2 * CH], F16, tag="wqk")
            # m=0 slice of k=0 first: unblocks the very first matmul sooner
            nc.scalar.dma_start(out=wqk_t[:, 0, 0:128], in_=wqk[0:128, 0:128])
            nc.scalar.dma_start(out=wqk_t[:, 0, 128:], in_=wqk[0:128, 128:])
            wqk_sb = [wqk_t[:, k, :] for k in range(NK)]
            bqk_sb = p_const.tile([128, 4], F32, tag="bqk")
            bv_sb = p_const.tile([128, VC], F32, tag="bv")
            tri2_sb = p_const.tile([128, 2, 128], F16, tag="tri2")
            wv_t = p_wv.tile([128, NK, VC], F16, tag="wv")
            wv_sb = [wv_t[:, k, :] for k in range(NK)]
            wp_t = p_wp.tile([128, 2, C], F16, tag="wp")
            wp_sb = [wp_t[:, c, :] for c in range(2)]

            # sync queue; xT k=0 split so its first Tq chunk lands first
            nc.sync.dma_start(out=xT_sb[0][:, 0:512], in_=xT[0:128, 0:512])
            nc.sync.dma_start(out=xT_sb[0][:, 512:], in_=xT[0:128, 512:])
            for k in (1, 3, 4):
                nc.sync.dma_start(out=xT_sb[k], in_=load_xT(k))
            # scalar queue (wqk k=0 already first); wqk staged so each
            # k-group lands just before the k-major QKV loop consumes it
            nc.scalar.dma_start(out=bqk_sb, in_=bqk[:, :])
            nc.scalar.dma_start(
                out=wqk_t[:, 1:4, :],
                in_=wqk[128:512, :].rearrange("(k p) c -> p k c", k=3),
            )
            nc.scalar.dma_start(
                out=wqk_t[:, 4:NK, :],
                in_=wqk[512:, :].rearrange("(k p) c -> p k c", k=NK - 4),
            )
            nc.scalar.dma_start(out=bv_sb, in_=bv[:, :])
            nc.scalar.dma_start(out=tri2_sb, in_=tri2[:, :, :])
            # gpsimd queue (software DGE)
            nc.gpsimd.dma_start(
                out=wv_t, in_=wv.rearrange("(k p) c -> p k c", k=NK)
            )
            for k in (2, 5, 6, 7):
                nc.gpsimd.dma_start(out=xT_sb[k], in_=load_xT(k))
            nc.gpsimd.dma_start(
                out=wp_t, in_=wp.rearrange("(c p) d -> p c d", c=2)
            )

            # q/k head-pair tiles [128, T] fp16: partitions 0:64 = even head,
            # 64:128 = odd head.  0,1 = q pairs; 2,3 = k pairs.
            qk_sb = [
                p_qk.tile([128, T], F16, tag="qk", name=f"qk{i}") for i in range(4)
            ]
            v_sb = [
                p_v.tile([128, G, HS + 1], F16, tag="v", name=f"v{i}")
                for i in range(NT)
            ]
            yT_sb = [
                p_yT.tile([128, T], F16, tag="yT", name=f"yT{i}") for i in range(2)
            ]

            def qk_chunk(m, j):
                """q (m=0,1) / k (m=2,3) projection for Tq chunk j."""
                ps = ps_mm.tile([128, 512], F32, tag="mm", name="ps_qk")
                for k in range(NK):
                    nc.tensor.matmul(
                        ps,
                        lhsT=wqk_sb[k][:, m * 128:(m + 1) * 128],
                        rhs=xT_sb[k][:, j * 512:(j + 1) * 512],
                        start=(k == 0),
                        stop=(k == NK - 1),
                    )
                nc.vector.tensor_scalar_add(
                    out=qk_sb[m][:, j * 512:(j + 1) * 512],
                    in0=ps,
                    scalar1=bqk_sb[:, m:m + 1],
                )

            def v_tile(t):
                """v projection for Tk tile t; bias + per-head ones column
                come from the host-prearranged wv/bv layout."""
                ps = ps_mm.tile([128, 512], F32, tag="mm", name="ps_v")
                for k in range(NK):
                    nc.tensor.matmul(
                        ps[:, 0:VC],
                        lhsT=xT_sb[k][:, t * 128:(t + 1) * 128],
                        rhs=wv_sb[k],
                        start=(k == 0),
                        stop=(k == NK - 1),
                    )
                vt = v_sb[t]
                nc.vector.tensor_add(
                    out=vt[:, :, :],
                    in0=ps[:, 0:VC].rearrange("p (g d) -> p g d", g=G),
                    in1=bv_sb.rearrange("p (g d) -> p g d", g=G),
                )

            def attn_chunk(c, j, popf, tail=False, after_quarter=None):
                """Attention for head pair c (heads 2c, 2c+1), Tq chunk j.
                The two sub-heads' score blocks live in the two banks of one
                [128, 2, 512] PSUM tile (one legal accumulation group per
                bank), sharing a single merged exp on ACT.  popf() emits one
                pure-PE filler unit (next-chunk QKV / prev-chunk proj)."""
                qp = qk_sb[c]
                kp = qk_sb[2 + c]
                nblk = 4 * j + 4
                psy_s = [
                    ps_y.tile([HS + 1, 512], F32, tag="y", name=f"psy{s}")
                    for s in range(2)
                ]

                def st_exp(i):
                    r = i - 4 * j
                    c0 = max(0, r) * 128  # first causally-valid column
                    pss = ps_st.tile([128, 2, 512], F32, tag="st", name="ps_s")
                    for s in range(2):  # sub-head at partitions 64s:64s+64
                        p0 = HS * s
                        nc.tensor.matmul(
                            pss[:, s, c0:512],
                            lhsT=kp[p0:p0 + HS, i * 128:(i + 1) * 128],
                            rhs=qp[p0:p0 + HS, j * 512 + c0:(j + 1) * 512],
                            start=True,
                            stop=True,
                        )
                    pt = p_pt.tile([128, 2, 512], F16, tag="pt")
                    nc.scalar.activation(
                        out=pt[:, :, c0:512],
                        in_=pss[:, :, c0:512],
                        func=mybir.ActivationFunctionType.Exp,
                        scale=float(1.0 / np.sqrt(HS)),
                    )
                    if r >= 0:
                        # zero the masked (k > q) entries of the diagonal
                        # sub-block post-exp; fp16 SBUF-only -> fast DVE mode
                        nc.vector.tensor_mul(
                            out=pt[:, :, c0:c0 + 128],
                            in0=pt[:, :, c0:c0 + 128],
                            in1=tri2_sb,
                        )
                    return c0, pt

                def pv(i, c0, pt):
                    for s in range(2):
                        nc.tensor.matmul(
                            psy_s[s][:, c0:512],
                            lhsT=v_sb[i][:, 2 * c + s, :],
                            rhs=pt[:, s, c0:512],
                            start=(i == 0),
                            stop=(i == nblk - 1),
                        )

                def norm_quarter(q):
                    """Normalize columns 128q:128(q+1) once their PV
                    accumulation group closed (tail chunk only)."""
                    qlo = 128 * q
                    col = j * 512 + qlo
                    for s in range(2):
                        rc = p_rc.tile([1, 128], F32, tag="rcq")
                        nc.vector.reciprocal(
                            out=rc, in_=psy_s[s][HS:HS + 1, qlo:qlo + 128]
                        )
                        bc = p_bc.tile([HS, 128], F32, tag="bcq")
                        nc.gpsimd.partition_broadcast(bc, rc)
                        nc.vector.tensor_mul(
                            out=yT_sb[c][HS * s:HS * s + HS, col:col + 128],
                            in0=psy_s[s][0:HS, qlo:qlo + 128],
                            in1=bc,
                        )
                    if after_quarter is not None:
                        after_quarter(q)

                def pv_tail(i, c0, pt):
                    """PV for the drain pair: the 4 diagonal blocks close one
                    128-column quarter each, so normalization and the output
                    projection of finished columns pipeline with the
                    remaining attention blocks."""
                    r = i - 4 * j
                    if r < 0:
                        for s in range(2):
                            nc.tensor.matmul(
                                psy_s[s][:, 0:512],
                                lhsT=v_sb[i][:, 2 * c + s, :],
                                rhs=pt[:, s, 0:512],
                                start=(i == 0),
                                stop=False,
                            )
                        return
                    for s in range(2):
                        for q in range(r, 4):
                            qlo = 128 * q
                            nc.tensor.matmul(
                                psy_s[s][:, qlo:qlo + 128],
                                lhsT=v_sb[i][:, 2 * c + s, :],
                                rhs=pt[:, s, qlo:qlo + 128],
                                start=False,
                                stop=(q == r),
                                skip_group_check=True,
                            )
                    norm_quarter(r)

                # one-block software pipeline: ST/exp of block i+1 is queued
                # on PE before PV of block i, so PE never idles on ACT.
                prev = None
                for i in range(nblk):
                    cur = (i, *st_exp(i))
                    if prev is not None:
                        (pv_tail if tail else pv)(*prev)
                    popf()
                    prev = cur
                (pv_tail if tail else pv)(*prev)
                popf()
                if tail:
                    return
                # normalization: 1/denom (row HS) broadcast down the
                # partitions on Pool, multiply straight out of PSUM.
                rcs = []
                for s in range(2):
                    rc = p_rc.tile([1, 512], F32, tag="rc")
                    nc.vector.reciprocal(out=rc, in_=psy_s[s][HS:HS + 1, :])
                    rcs.append(rc)
                bcs = []
                for s in range(2):
                    bc = p_bc.tile([HS, 512], F32, tag="bc")
                    nc.gpsimd.partition_broadcast(bc, rcs[s])
                    bcs.append(bc)
                popf()
                for s in range(2):
                    nc.vector.tensor_mul(
                        out=yT_sb[c][
                            HS * s:HS * s + HS, j * 512:(j + 1) * 512
                        ],
                        in0=psy_s[s][0:HS, :],
                        in1=bcs[s],
                    )

            def proj_tile(t, tail=False, act_copy=False):
                """Output projection for Tq tile t.  Tail tiles skip the SBUF
                staging copy entirely: a gpsimd-issued casting DMA streams the
                PSUM accumulators straight to DRAM (Pool is idle in the
                drain, and this keeps DVE off the critical path)."""
                osb = p_out.tile([128, C], F16, tag="os")
                for o in range(2):
                    ps = ps_mm.tile([128, 512], F32, tag="mm", name="ps_o")
                    for cc in range(2):
                        nc.tensor.matmul(
                            ps,
                            lhsT=yT_sb[cc][:, t * 128:(t + 1) * 128],
                            rhs=wp_sb[cc][:, o * 512:(o + 1) * 512],
                            start=(cc == 0),
                            stop=(cc == 1),
                        )
                    on_act = act_copy == "act" or (act_copy == "split" and o == 1)
                    if on_act:
                        nc.scalar.copy(
                            out=osb[:, o * 512:(o + 1) * 512], in_=ps
                        )
                    else:
                        nc.vector.tensor_copy(
                            out=osb[:, o * 512:(o + 1) * 512], in_=ps
                        )
                if tail:
                    # half-stores issue as soon as each eviction lands
                    nc.sync.dma_start(
                        out=out[t * 128:(t + 1) * 128, 0:512],
                        in_=osb[:, 0:512],
                    )
                    nc.scalar.dma_start(
                        out=out[t * 128:(t + 1) * 128, 512:C],
                        in_=osb[:, 512:C],
                    )
                else:
                    nc.sync.dma_start(
                        out=out[t * 128:(t + 1) * 128, :], in_=osb
                    )

            def qkv0():
                """Chunk-0 QKV, k-major: all four q/k accumulators (and then
                all four v tiles) advance together so each arriving xT k-tile
                unlocks 4 matmuls, hiding the initial DMA latency.  The two
                extra accumulators borrow an idle ps_st tile's banks."""
                ms = (0, 2, 1, 3)
                psa = ps_mm.tile([128, 512], F32, tag="mm", name="ps_qa")
                psb = ps_mm.tile([128, 512], F32, tag="mm", name="ps_qb")
                pscd = ps_st.tile([128, 2, 512], F32, tag="st", name="ps_qcd")
                accs = [psa, psb, pscd[:, 0, :], pscd[:, 1, :]]
                for k in range(NK):
                    for mi, m in enumerate(ms):
                        nc.tensor.matmul(
                            accs[mi],
                            lhsT=wqk_sb[k][:, m * 128:(m + 1) * 128],
                            rhs=xT_sb[k][:, 0:512],
                            start=(k == 0),
                            stop=(k == NK - 1),
                        )
                for mi, m in enumerate(ms):
                    nc.vector.tensor_scalar_add(
                        out=qk_sb[m][:, 0:512],
                        in0=accs[mi],
                        scalar1=bqk_sb[:, m:m + 1],
                    )
                psa2 = ps_mm.tile([128, 512], F32, tag="mm", name="ps_va")
                psb2 = ps_mm.tile([128, 512], F32, tag="mm", name="ps_vb")
                pscd2 = ps_st.tile([128, 2, 512], F32, tag="st", name="ps_vcd")
                vaccs = [
                    psa2[:, 0:VC],
                    psb2[:, 0:VC],
                    pscd2[:, 0, 0:VC],
                    pscd2[:, 1, 0:VC],
                ]
                for k in range(NK):
                    for t in range(4):
                        nc.tensor.matmul(
                            vaccs[t],
                            lhsT=xT_sb[k][:, t * 128:(t + 1) * 128],
                            rhs=wv_sb[k],
                            start=(k == 0),
                            stop=(k == NK - 1),
                        )
                for t in range(4):
                    nc.vector.tensor_add(
                        out=v_sb[t][:, :, :],
                        in0=vaccs[t].rearrange("p (g d) -> p g d", g=G),
                        in1=bv_sb.rearrange("p (g d) -> p g d", g=G),
                    )

            # ---- schedule ----
            # chunk 0's QKV up front; then per chunk j: both attention pairs
            # with next-chunk QKV + prev-chunk proj interleaved as fillers;
            # the final chunk's proj drains at the end.
            def body():
                qkv0()
                for j in range(NCHUNK):
                    # urgent units are emitted one-per-pop before the evenly
                    # spread ones; chunk 3's own v tiles must land before
                    # their PV consumers in the same chunk.
                    urgent = []
                    units = []
                    if j + 1 < NCHUNK:
                        units += [
                            lambda m=m, j1=j + 1: qk_chunk(m, j1)
                            for m in (0, 2, 1, 3)
                        ]
                        if j + 1 < NCHUNK - 1:
                            units += [
                                lambda t=t: v_tile(t)
                                for t in range(4 * j + 4, 4 * j + 8)
                            ]
                        else:  # last chunk's v tiles become its own fillers
                            units += [
                                lambda t=t: v_tile(t)
                                for t in range(4 * j + 4, 4 * j + 6)
                            ]
                            urgent_next = [
                                lambda t=t: v_tile(t)
                                for t in range(4 * j + 6, 4 * j + 8)
                            ]
                    if j == NCHUNK - 1:
                        urgent += urgent_next
                    if j > 0:
                        units += [
                            lambda t=t: proj_tile(t)
                            for t in range(4 * (j - 1), 4 * j)
                        ]
                    # pop points: one per block plus 2 extra per pair
                    npop = 2 * (4 * j + 4) + 4
                    emitted = [0]
                    calls = [0]
                    nunits = len(units)

                    def popf():
                        if urgent:
                            urgent.pop(0)()
                            return
                        calls[0] += 1
                        while units and emitted[0] * npop < calls[0] * nunits:
                            u = units.pop(0)
                            u()
                            emitted[0] += 1

                    last = j == NCHUNK - 1
                    attn_chunk(0, j, popf)
                    attn_chunk(
                        1,
                        j,
                        popf,
                        tail=last,
                        after_quarter=(
                            (
                                lambda q: proj_tile(
                                    4 * j + q,
                                    tail=True,
                                    act_copy=(
                                        "split"
                                        if q == 3
                                        else ("act" if q == 2 else False)
                                    ),
                                )
                            )
                            if last
                            else None
                        ),
                    )
                    while urgent:
                        urgent.pop(0)()
                    while units:
                        units.pop(0)()

            if repeat == 1:
                body()
            else:  # benchmarking only: loop the whole compute on-device
                with tc.For_i(0, repeat, 1):
                    body()

    nc.finalize()
    return nc


_NC = None


def _get_nc():
    global _NC
    if _NC is None:
        _NC = _build_nc()
    return _NC


def _make_in_maps(x, w_attn, b_attn, w_proj):
    tri = (np.arange(128)[:, None] <= np.arange(128)[None, :]).astype(np.float16)
    tri2 = np.stack([tri, tri], axis=1)  # [128, 2, 128], one copy per bank
    in_maps = []
    for core in range(8):
        b, g = divmod(core, 4)
        c0 = CH * g
        xTb = np.ascontiguousarray(x[b].T).astype(np.float16)
        wqk = np.concatenate(
            [w_attn[:, c0:c0 + CH], w_attn[:, C + c0:C + c0 + CH]], axis=1
        ).astype(np.float16)
        # v weights/bias with a zero/one ones-column per head
        wv_flat = w_attn[:, 2 * C + c0:2 * C + c0 + CH]  # [C, CH]
        wv = np.zeros((C, VC), dtype=np.float16)
        bv = np.zeros((128, VC), dtype=np.float32)
        bvec = b_attn[2 * C + c0:2 * C + c0 + CH]
        for gg in range(G):
            wv[:, gg * (HS + 1):gg * (HS + 1) + HS] = wv_flat[
                :, gg * HS:(gg + 1) * HS
            ]
            bv[:, gg * (HS + 1):gg * (HS + 1) + HS] = bvec[
                None, gg * HS:(gg + 1) * HS
            ]
            bv[:, gg * (HS + 1) + HS] = 1.0
        wp = np.ascontiguousarray(w_proj[c0:c0 + CH, :]).astype(np.float16)
        bqk = np.concatenate(
            [b_attn[c0:c0 + CH], b_attn[C + c0:C + c0 + CH]]
        ).reshape(4, 128).T.astype(np.float32)
        bqk = np.ascontiguousarray(bqk)
        in_maps.append(
            {
                "xT": xTb,
                "wqk": wqk,
                "wv": wv,
                "wp": wp,
                "bqk": bqk,
                "bv": bv,
                "tri2": tri2,
            }
        )
    return in_maps


def kernel(x, w_attn, b_attn, w_proj, b_proj, trace=False):
    global LAST_RESULT
    x = np.asarray(x, dtype=np.float32)
    w_attn = np.asarray(w_attn, dtype=np.float32)
    b_attn = np.asarray(b_attn, dtype=np.float32)
    w_proj = np.asarray(w_proj, dtype=np.float32)
    b_proj = np.asarray(b_proj, dtype=np.float32)

    nc = _get_nc()
    in_maps = _make_in_maps(x, w_attn, b_attn, w_proj)
    res = bass_utils.run_bass_kernel_spmd(
        nc, in_maps, core_ids=list(range(8)), trace=trace
    )
    LAST_RESULT = res
    parts = [r["out"] for r in res.results]
    out = np.empty((B, T, C), dtype=np.float32)
    for b in range(B):
        acc = parts[4 * b].astype(np.float32)
        for g in range(1, 4):
            acc = acc + parts[4 * b + g].astype(np.float32)
        out[b] = acc + b_proj[None, :]
    return out
